# revision 25
# baseline (speedup 1.0000x reference)
"""Trainium2 Bass kernel for nn_MultiHeadDotProductAttention_24756191494231.

Masked (toeplitz-structured) linear attention:
    q = relu(query/8); k = relu(key)
    attn = (q @ k^T) * |toeplitz_mask| ; attn /= attn.sum(-1) ; out = attn @ v

Sharding: 8 cores = 2 batch-groups (4 batches) x 4 head-groups (3 heads).
Each core computes 12 (batch, head) pairs.

Host packs q/k (relu+scale folded, bf16), V with a ones column per k-chunk
(the AV matmul's 65th column then yields the row-sum Z), and the full
per-head |toeplitz| mask laid out exactly like the device A tile.

Device pipeline per (head, batch) pair, software-pipelined one slot deep
(masks of pair i overlap the AV matmuls of pair i-1 on the PE):
  S^T[k,l] = K'^T.T @ Q'^T          (bf16 matmuls into PSUM, 5 k-chunks)
  A[k,l]   = S^T * msk              split across engines per chunk
  O[l,:]   = A.T @ [V|1]            (bf16; ones column gives Z = row-sum)
  out      = O[:, :64] / Z          (reciprocal + multiply, bf16 DMA out)
"""
import sys

for _p in ("/opt/trn_rl_repo", "/root/.axon_site/_ro/trn_rl_repo"):
    if _p not in sys.path:
        sys.path.insert(0, _p)

import numpy as np
import ml_dtypes

NBX = NBY = 24
B, H, D = 8, 12, 64
L = NBX * NBY + 1          # 577
LP = 578                   # chunk stride in A/msk tiles (even => 4B aligned)
NB = 4                     # batches per core
NH = 3                     # heads per core
CNT = [121, 120, 120, 120, 96]       # k-chunk sizes (CLS + 24-aligned grid)
KS = [0, 121, 241, 361, 481]         # k-chunk starts
LW = [128, 128, 128, 128, 65]        # l-chunk sizes for the AV matmuls

# --- tuning knobs ---
PC = 380                   # tail cols of chunks 1..3 multiplied on GpSimd
A4 = 0                     # head cols of chunk 4 via ScalarE copy + DVE mult
AV_ORDER = [0, 4, 1, 2, 3]  # accumulation order of AV k-chunks
# PE-slot token sequence: Sc = S-matmul chunk c, Gk = AV group lc=k of the
# previous pair. Masks are emitted right after their S chunk.
SLOT_SEQ = ["S1", "S2", "S0", "S3", "G0", "S4", "G1", "G2", "G3", "G4"]
NORM_MODE = "dve"          # "dve" | "act_pool" | "act_dve"
AT_BUFS = 3
PC0 = 380                  # pool share for the first two (fill) slots
POOL_MERGE12 = False       # merge the c1+c2 pool tails into one op
WARM = 0                   # dummy matmuls to pre-ramp the PE during loads
SPLIT_LAST_STORE = False   # pipeline the last pair's normalize/store
LAST_STORE_SWDGE = False   # issue the final store through gpsimd SWDGE
FIRST_Q_SWDGE = False      # first q load via SWDGE (parallel DGE gen)
M13 = False                # chunks 1+3 share one PSUM tile, merged Act copy
RECIP_ON_ACT = False       # reciprocal on the Activation engine
PC_ODD = 380               # pool share on odd slots (smooths phase bunching)
SLOT0_SEQ = SLOT_SEQ       # fill-slot S order (no AV work in slot 0)

_CACHE = {}


def _split_excess_waits(nc):
    """Walrus accepts at most ONE sync-wait per instruction (zero on
    Pool-engine ops). Move excess waits onto same-engine InstEventSemaphore
    instructions inserted immediately before the offending instruction."""
    import concourse.mybir as mb
    ctr = 0
    f = nc.m.functions[0]
    for bb in f.blocks:
        insts = list(bb.instructions)
        out = []
        changed = False
        for inst in insts:
            si = inst.sync_info
            keep = 0 if inst.engine == mb.EngineType.Pool else 1
            if si is not None and len(si.on_wait) > keep:
                waits = list(si.on_wait)
                moved = waits[:-keep] if keep else waits
                kept = waits[-keep:] if keep else []
                for w in moved:
                    ctr += 1
                    ev = mb.InstEventSemaphore(
                        name=f"zz_waitsplit_{ctr}", ins=[], outs=[])
                    ev.engine = inst.engine
                    ev.sync_info = mb.SyncInfo(on_wait=[w], on_update=[])
                    out.append(ev)
                inst.sync_info = mb.SyncInfo(
                    on_wait=kept, on_update=list(si.on_update))
                changed = True
            out.append(inst)
        if changed:
            bb.instructions = out


def _build_bass():
    import concourse.bass as bass
    import concourse.mybir as mybir
    from concourse.bass_types import AP
    from concourse.tile import TileContext

    F32 = mybir.dt.float32
    BF16 = mybir.dt.bfloat16
    Alu = mybir.AluOpType
    Act = mybir.ActivationFunctionType

    nc = bass.Bass("TRN2")
    k_d = nc.dram_tensor("kt", (NH, 128, 2 * L), BF16, kind="ExternalInput")
    q_d = nc.dram_tensor("qt", (NH, 128, 2 * L), BF16, kind="ExternalInput")
    v_d = nc.dram_tensor("v5", (NH, 128, NB * 325), BF16, kind="ExternalInput")
    m_d = nc.dram_tensor("msk", (NH, 128, 5 * LP), BF16, kind="ExternalInput")
    o_d = nc.dram_tensor("o", (NH, 128, NB * 320), BF16, kind="ExternalOutput")

    with TileContext(nc) as tc:
        with (
            tc.tile_pool(name="sb", bufs=2) as sb,
            tc.tile_pool(name="ps", bufs=3, space="PSUM") as ps,
            tc.tile_pool(name="ps_o", bufs=2, space="PSUM") as ps_o,
        ):
            def load_head(h, split_first):
                kT = sb.tile([128, 2 * L], BF16, tag="kT")
                qT = sb.tile([128, 2 * L], BF16, tag="qT")
                msk = sb.tile([128, 5 * LP], BF16, tag="msk")
                v5 = sb.tile([128, NB * 325], BF16, tag="v5")
                if split_first:
                    nc.sync.dma_start(kT[0:64, :], k_d[h, 0:64, :])
                    if FIRST_Q_SWDGE:
                        # SWDGE generation runs on the (idle) Pool engine in
                        # parallel with the HWDGE generation of the k load
                        nc.gpsimd.dma_start(qT[0:64, :], q_d[h, 0:64, :])
                    else:
                        nc.sync.dma_start(qT[0:64, :], q_d[h, 0:64, :])
                    nc.sync.dma_start(msk[:, 0:LP], m_d[h, :, 0:LP])
                    nc.sync.dma_start(kT[64:128, :], k_d[h, 64:128, :])
                    nc.sync.dma_start(qT[64:128, :], q_d[h, 64:128, :])
                    for c in range(1, 5):
                        nc.sync.dma_start(msk[:, LP * c:LP * (c + 1)],
                                          m_d[h, :, LP * c:LP * (c + 1)])
                else:
                    nc.sync.dma_start(kT, k_d[h])
                    nc.sync.dma_start(qT, q_d[h])
                    nc.sync.dma_start(msk, m_d[h])
                nc.sync.dma_start(v5, v_d[h])
                o_sb = sb.tile([128, NB * 320], BF16, tag="o_sb")
                return dict(kT=kT, qT=qT, msk=msk, v5=v5, o_sb=o_sb, h=h)

            def s_chunk(R, b, c, dest=None, off=0):
                # dest/off: write into a shared tile at a column offset
                # (matmul pieces never cross a 512-f32 PSUM bank boundary)
                pr = 64 * (b // 2)
                xo = L * (b % 2)
                cnt = CNT[c]
                if dest is None:
                    sp = ps.tile([128, LP], F32, tag="sp",
                                 bufs=(2 if M13 else 3))
                else:
                    sp = dest
                bk = 512 * ((off + 511) // 512)
                if bk == off:
                    bk = off + 512
                lhs = R["kT"][pr:pr + 64, xo + KS[c]:xo + KS[c] + cnt]
                nc.tensor.matmul(sp[0:cnt, off:bk], lhs,
                                 R["qT"][pr:pr + 64, xo:xo + (bk - off)],
                                 start=True, stop=True)
                nc.tensor.matmul(sp[0:cnt, bk:off + L], lhs,
                                 R["qT"][pr:pr + 64, xo + (bk - off):xo + L],
                                 start=True, stop=True)
                return sp

            def mask_chunk(R, a_t, sp, c):
                # emit the PSUM->SBUF path for chunk c right after its S
                if c == 0:
                    nc.vector.tensor_tensor(
                        out=a_t[0:121, 0:L], in0=sp[0:121, 0:L],
                        in1=R["msk"][0:121, 0:L], op=Alu.mult)
                elif c in (1, 2, 3):
                    nc.scalar.activation(a_t[0:120, LP * c:LP * c + L],
                                         sp[0:120, 0:L], Act.Copy)
                else:
                    if A4 > 0:
                        nc.scalar.activation(a_t[0:96, 4 * LP:4 * LP + A4],
                                             sp[0:96, 0:A4], Act.Copy)
                    nc.vector.tensor_tensor(
                        out=a_t[0:96, 4 * LP + A4:4 * LP + L],
                        in0=sp[0:96, A4:L],
                        in1=R["msk"][0:96, 4 * LP + A4:4 * LP + L],
                        op=Alu.mult)

            def av_group(Rj, j_b, a_t, o_ps, lc):
                for idx, c in enumerate(AV_ORDER):
                    nc.tensor.matmul(
                        o_ps[0:LW[lc], 65 * lc:65 * lc + 65],
                        a_t[0:CNT[c], LP * c + 128 * lc:LP * c + 128 * lc + LW[lc]],
                        Rj["v5"][0:CNT[c], 325 * j_b + 65 * c:325 * j_b + 65 * c + 65],
                        start=(idx == 0), stop=(idx == 4))

            def late_mults(R, a_t, pc):
                # mask multiplies for the Act-copied chunks of pair i
                io_h = a_t[0:120, LP:4 * LP].rearrange(
                    "p (c l) -> p c l", l=LP)[:, :, 0:L - pc]
                mk_h = R["msk"][0:120, LP:4 * LP].rearrange(
                    "p (c l) -> p c l", l=LP)[:, :, 0:L - pc]
                nc.vector.tensor_tensor(out=io_h, in0=io_h, in1=mk_h,
                                        op=Alu.mult)
                if pc > 0:
                    if POOL_MERGE12:
                        io12 = a_t[0:120, LP:3 * LP].rearrange(
                            "p (c l) -> p c l", l=LP)[:, :, L - pc:L]
                        mk12 = R["msk"][0:120, LP:3 * LP].rearrange(
                            "p (c l) -> p c l", l=LP)[:, :, L - pc:L]
                        nc.gpsimd.tensor_tensor(out=io12, in0=io12, in1=mk12,
                                                op=Alu.mult)
                        cs = (3,)
                    else:
                        cs = (1, 2, 3)
                    for c in cs:
                        io_t = a_t[0:120, LP * c + L - pc:LP * c + L]
                        nc.gpsimd.tensor_tensor(
                            out=io_t, in0=io_t,
                            in1=R["msk"][0:120, LP * c + L - pc:LP * c + L],
                            op=Alu.mult)
                if A4 > 0:
                    io4 = a_t[0:96, 4 * LP:4 * LP + A4]
                    nc.vector.tensor_tensor(
                        out=io4, in0=io4,
                        in1=R["msk"][0:96, 4 * LP:4 * LP + A4], op=Alu.mult)

            def finish_pair(Rj, j_b, o_psj, last):
                rz = sb.tile([128, 5], F32, tag="rz")
                zin = o_psj[:, :].rearrange(
                    "p (c d) -> p c d", d=65)[:, :, 64:65]
                if RECIP_ON_ACT and not last:
                    nc.scalar.activation(
                        rz[:, :].rearrange("p (c d) -> p c d", d=1), zin,
                        Act.Reciprocal)
                else:
                    nc.vector.reciprocal(
                        rz[:, :].rearrange("p (c d) -> p c d", d=1), zin)
                in0 = o_psj[:, :].rearrange(
                    "p (c d) -> p c d", d=65)[:, :, 0:64]
                rzb = AP(rz.tensor, 0, [[5, 128], [1, 5], [0, 64]])
                out_ap = Rj["o_sb"][:, 320 * j_b:320 * j_b + 320].rearrange(
                    "p (c d) -> p c d", d=64)
                if NORM_MODE in ("act_pool", "act_dve") and not last:
                    o_c = sb.tile([128, 320], BF16, tag="o_c")
                    nc.scalar.activation(
                        o_c[:, :].rearrange("p (c d) -> p c d", d=64), in0,
                        Act.Copy)
                    eng = nc.gpsimd if NORM_MODE == "act_pool" else nc.vector
                    eng.tensor_tensor(
                        out=out_ap,
                        in0=o_c[:, :].rearrange("p (c d) -> p c d", d=64),
                        in1=rzb, op=Alu.mult)
                else:
                    nc.vector.tensor_tensor(out=out_ap, in0=in0, in1=rzb,
                                            op=Alu.mult)
                eng = nc.gpsimd if (last and LAST_STORE_SWDGE) else nc.sync
                eng.dma_start(
                    o_d[Rj["h"], :, 320 * j_b:320 * j_b + 320],
                    Rj["o_sb"][:, 320 * j_b:320 * j_b + 320])

            pend = None            # (Rj, j_b, a_t_j, o_ps_j)

            pairs = [(h, b) for h in range(NH) for b in range(NB)]
            heads_loaded = [False] * NH
            R_by_head = {}

            def ensure_head(h):
                if not heads_loaded[h]:
                    R_by_head[h] = load_head(h, split_first=(h == 0))
                    heads_loaded[h] = True
                return R_by_head[h]

            if WARM:
                # dummy matmuls keep the PE busy while the first loads are
                # in flight, so the real matmuls start at full p-state
                dsrc = sb.tile([1, 326], BF16, tag="dsrc", bufs=1)
                nc.vector.memset(dsrc[:, :], 0.0)
                dps = ps_o.tile([128, 325], F32, tag="o_ps")
                for _ in range(WARM):
                    nc.tensor.matmul(dps[0:1, 0:325], dsrc[0:1, 0:1],
                                     dsrc[0:1, 0:325], start=True, stop=True)

            ensure_head(0)

            for s, (h, b) in enumerate(pairs):
                R = R_by_head[h]
                if b == 2 and h + 1 < NH:
                    ensure_head(h + 1)

                a_t = sb.tile([128, 5 * LP], BF16, tag="a_t", bufs=AT_BUFS)
                if pend is not None:
                    Rj, j_b, a_tj, o_psj = pend

                m13 = None
                for tok in (SLOT0_SEQ if s == 0 else SLOT_SEQ):
                    if tok[0] == "S":
                        c = int(tok[1])
                        if M13 and c in (1, 3):
                            if m13 is None:
                                m13 = ps.tile([128, 2 * LP], F32, tag="m13",
                                              bufs=1)
                            s_chunk(R, b, c, dest=m13,
                                    off=(0 if c == 1 else LP))
                            if c == 3:
                                # one merged ScalarE copy for chunks 1 and 3
                                nc.scalar.activation(
                                    a_t[0:120, LP:5 * LP].rearrange(
                                        "p (c l) -> p c l",
                                        l=2 * LP)[:, :, 0:L],
                                    m13[0:120, :].rearrange(
                                        "p (c l) -> p c l", l=LP)[:, :, 0:L],
                                    Act.Copy)
                        else:
                            sp = s_chunk(R, b, c)
                            mask_chunk(R, a_t, sp, c)
                    else:
                        if pend is not None:
                            av_group(Rj, j_b, a_tj, o_psj, int(tok[1]))

                if pend is not None:
                    finish_pair(Rj, j_b, o_psj, last=False)

                late_mults(R, a_t, PC0 if s < 2 else (PC if s % 2 == 0 else PC_ODD))

                o_ps = ps_o.tile([128, 325], F32, tag="o_ps",
                                 bufs=(1 if M13 else 2))
                pend = (R, b, a_t, o_ps)

            # ---- drain ----
            Rj, j_b, a_tj, o_psj = pend
            if SPLIT_LAST_STORE:
                for lc in range(4):
                    av_group(Rj, j_b, a_tj, o_psj, lc)
                rz = sb.tile([128, 5], F32, tag="rz")
                o5 = o_psj[:, :].rearrange("p (c d) -> p c d", d=65)
                nc.vector.reciprocal(
                    rz[:, 0:4].rearrange("p (c d) -> p c d", d=1),
                    o5[:, 0:4, 64:65])
                out5 = Rj["o_sb"][:, 320 * j_b:320 * j_b + 320].rearrange(
                    "p (c d) -> p c d", d=64)
                nc.vector.tensor_tensor(
                    out=out5[:, 0:4, :], in0=o5[:, 0:4, 0:64],
                    in1=AP(rz.tensor, 0, [[5, 128], [1, 4], [0, 64]]),
                    op=Alu.mult)
                nc.sync.dma_start(
                    o_d[Rj["h"], :, 320 * j_b:320 * j_b + 256],
                    Rj["o_sb"][:, 320 * j_b:320 * j_b + 256])
                av_group(Rj, j_b, a_tj, o_psj, 4)
                nc.vector.reciprocal(
                    rz[:, 4:5].rearrange("p (c d) -> p c d", d=1),
                    o5[:, 4:5, 64:65])
                nc.vector.tensor_tensor(
                    out=out5[:, 4:5, :], in0=o5[:, 4:5, 0:64],
                    in1=AP(rz.tensor, 4, [[5, 128], [1, 1], [0, 64]]),
                    op=Alu.mult)
                nc.sync.dma_start(
                    o_d[Rj["h"], :, 320 * j_b + 256:320 * j_b + 320],
                    Rj["o_sb"][:, 320 * j_b + 256:320 * j_b + 320])
            else:
                for lc in range(5):
                    av_group(Rj, j_b, a_tj, o_psj, lc)
                finish_pair(Rj, j_b, o_psj, last=True)

    _split_excess_waits(nc)
    return nc


def _get_nc():
    if "nc" not in _CACHE:
        _CACHE["nc"] = _build_bass()
    return _CACHE["nc"]


def _dist_index():
    if "dist" not in _CACHE:
        gi = np.arange(NBX)
        gj = np.arange(NBY)
        di = (gi[:, None, None, None] - gi[None, None, :, None] + NBX) * 2 * NBY
        dj = gj[None, :, None, None] - gj[None, None, None, :] + NBY
        _CACHE["dist"] = (di + dj).reshape(NBX * NBY, NBX * NBY)
    return _CACHE["dist"]


def _host_shard(query, key, value, topological_params):
    """Build the 8 per-core input dicts (slicing / layout / packing)."""
    q = np.asarray(query, dtype=np.float32)
    k = np.asarray(key, dtype=np.float32)
    v = np.asarray(value, dtype=np.float32)
    p = np.asarray(topological_params, dtype=np.float32)

    qs = np.maximum(q * 0.125, 0.0)
    ks = np.maximum(k, 0.0)

    dist = _dist_index()
    absp = np.abs(p)
    msk_all = np.zeros((H, 128, 5 * LP), dtype=ml_dtypes.bfloat16)
    for h in range(H):
        M = np.ones((L, L), dtype=np.float32)
        M[1:, 1:] = np.take(absp[h], dist)      # [q_grid, k_grid]
        MT = M.T                                # [k, l]
        for c in range(5):
            n = CNT[c]
            msk_all[h, 0:n, LP * c:LP * c + L] = MT[KS[c]:KS[c] + n, :]

    def pack_T(x, bs, hs):
        t = x[bs, :, hs, :]                       # [4, L, 3, 64]
        t = t.transpose(2, 0, 3, 1)               # [3, 4, 64, L]
        t = t.reshape(3, 2, 2, 64, L)             # [3, bhi, blo, d, L]
        t = t.transpose(0, 1, 3, 2, 4)            # [3, bhi, d, blo, L]
        return np.ascontiguousarray(
            t.reshape(3, 128, 2 * L)).astype(ml_dtypes.bfloat16)

    in_maps = []
    for u in range(2):            # batch group
        for g in range(4):        # head group
            bs = slice(4 * u, 4 * u + 4)
            hs = slice(3 * g, 3 * g + 3)
            vs = v[bs, :, hs, :]                  # [4, L, 3, 64]
            v_r = np.zeros((3, 128, NB, 5, 65), np.float32)
            for c in range(5):
                n = CNT[c]
                blk = vs[:, KS[c]:KS[c] + n].transpose(2, 1, 0, 3)
                v_r[:, :n, :, c, 0:64] = blk
                v_r[:, :n, :, c, 64] = 1.0
            in_maps.append({
                "kt": pack_T(ks, bs, hs),
                "qt": pack_T(qs, bs, hs),
                "v5": np.ascontiguousarray(
                    v_r.reshape(3, 128, NB * 325)).astype(ml_dtypes.bfloat16),
                "msk": np.ascontiguousarray(msk_all[hs]),
            })
    return in_maps


def kernel(query, key, value, topological_params):
    from concourse import bass_utils
    nc = _get_nc()
    in_maps = _host_shard(query, key, value, topological_params)
    res = bass_utils.run_bass_kernel_spmd(nc, in_maps, core_ids=list(range(8)))
    out = np.empty((B, L, H, D), dtype=np.float32)
    for u in range(2):
        for g in range(4):
            o = res.results[4 * u + g]["o"]          # [3, 128, NB*320] bf16
            o = o.astype(np.float32).reshape(3, 128, NB, 5, 64)
            for lc in range(5):
                lw = LW[lc]
                blk = o[:, 0:lw, :, lc, :]           # [3, lw, 4, 64]
                out[4 * u:4 * u + 4, 128 * lc:128 * lc + lw,
                    3 * g:3 * g + 3, :] = blk.transpose(2, 1, 0, 3)
    return out


# revision 26
# speedup vs baseline: 1.0039x; 1.0039x over previous
"""Trainium2 Bass kernel for nn_MultiHeadDotProductAttention_24756191494231.

Masked (toeplitz-structured) linear attention:
    q = relu(query/8); k = relu(key)
    attn = (q @ k^T) * |toeplitz_mask| ; attn /= attn.sum(-1) ; out = attn @ v

Sharding: 8 cores = 2 batch-groups (4 batches) x 4 head-groups (3 heads).
Each core computes 12 (batch, head) pairs.

Host packs q/k (relu+scale folded, bf16), V with a ones column per k-chunk
(the AV matmul's 65th column then yields the row-sum Z), and the full
per-head |toeplitz| mask laid out exactly like the device A tile.

Device pipeline per (head, batch) pair, software-pipelined one slot deep
(masks of pair i overlap the AV matmuls of pair i-1 on the PE):
  S^T[k,l] = K'^T.T @ Q'^T          (bf16 matmuls into PSUM, 5 k-chunks)
  A[k,l]   = S^T * msk              split across engines per chunk
  O[l,:]   = A.T @ [V|1]            (bf16; ones column gives Z = row-sum)
  out      = O[:, :64] / Z          (reciprocal + multiply, bf16 DMA out)
"""
import sys

for _p in ("/opt/trn_rl_repo", "/root/.axon_site/_ro/trn_rl_repo"):
    if _p not in sys.path:
        sys.path.insert(0, _p)

import numpy as np
import ml_dtypes

NBX = NBY = 24
B, H, D = 8, 12, 64
L = NBX * NBY + 1          # 577
LP = 578                   # chunk stride in A/msk tiles (even => 4B aligned)
NB = 4                     # batches per core
NH = 3                     # heads per core
CNT = [121, 120, 120, 120, 96]       # k-chunk sizes (CLS + 24-aligned grid)
KS = [0, 121, 241, 361, 481]         # k-chunk starts
LW = [128, 128, 128, 128, 65]        # l-chunk sizes for the AV matmuls

# --- tuning knobs ---
PC = 384                   # tail cols of chunks 1..3 multiplied on GpSimd
A4 = 0                     # head cols of chunk 4 via ScalarE copy + DVE mult
AV_ORDER = [0, 4, 1, 2, 3]  # accumulation order of AV k-chunks
# PE-slot token sequence: Sc = S-matmul chunk c, Gk = AV group lc=k of the
# previous pair. Masks are emitted right after their S chunk.
SLOT_SEQ = ["S1", "S2", "S0", "S3", "G0", "S4", "G1", "G2", "G3", "G4"]
NORM_MODE = "dve"          # "dve" | "act_pool" | "act_dve"
AT_BUFS = 3
PC0 = 380                  # pool share for the first two (fill) slots
POOL_MERGE12 = False       # merge the c1+c2 pool tails into one op
WARM = 0                   # dummy matmuls to pre-ramp the PE during loads
SPLIT_LAST_STORE = False   # pipeline the last pair's normalize/store
LAST_STORE_SWDGE = False   # issue the final store through gpsimd SWDGE
FIRST_Q_SWDGE = False      # first q load via SWDGE (parallel DGE gen)
M13 = False                # chunks 1+3 share one PSUM tile, merged Act copy
RECIP_ON_ACT = False       # reciprocal on the Activation engine
PC_ODD = 380               # pool share on odd slots (smooths phase bunching)
SLOT0_SEQ = SLOT_SEQ       # fill-slot S order (no AV work in slot 0)

_CACHE = {}


def _split_excess_waits(nc):
    """Walrus accepts at most ONE sync-wait per instruction (zero on
    Pool-engine ops). Move excess waits onto same-engine InstEventSemaphore
    instructions inserted immediately before the offending instruction."""
    import concourse.mybir as mb
    ctr = 0
    f = nc.m.functions[0]
    for bb in f.blocks:
        insts = list(bb.instructions)
        out = []
        changed = False
        for inst in insts:
            si = inst.sync_info
            keep = 0 if inst.engine == mb.EngineType.Pool else 1
            if si is not None and len(si.on_wait) > keep:
                waits = list(si.on_wait)
                moved = waits[:-keep] if keep else waits
                kept = waits[-keep:] if keep else []
                for w in moved:
                    ctr += 1
                    ev = mb.InstEventSemaphore(
                        name=f"zz_waitsplit_{ctr}", ins=[], outs=[])
                    ev.engine = inst.engine
                    ev.sync_info = mb.SyncInfo(on_wait=[w], on_update=[])
                    out.append(ev)
                inst.sync_info = mb.SyncInfo(
                    on_wait=kept, on_update=list(si.on_update))
                changed = True
            out.append(inst)
        if changed:
            bb.instructions = out


def _build_bass():
    import concourse.bass as bass
    import concourse.mybir as mybir
    from concourse.bass_types import AP
    from concourse.tile import TileContext

    F32 = mybir.dt.float32
    BF16 = mybir.dt.bfloat16
    Alu = mybir.AluOpType
    Act = mybir.ActivationFunctionType

    nc = bass.Bass("TRN2")
    k_d = nc.dram_tensor("kt", (NH, 128, 2 * L), BF16, kind="ExternalInput")
    q_d = nc.dram_tensor("qt", (NH, 128, 2 * L), BF16, kind="ExternalInput")
    v_d = nc.dram_tensor("v5", (NH, 128, NB * 325), BF16, kind="ExternalInput")
    m_d = nc.dram_tensor("msk", (NH, 128, 5 * LP), BF16, kind="ExternalInput")
    o_d = nc.dram_tensor("o", (NH, 128, NB * 320), BF16, kind="ExternalOutput")

    with TileContext(nc) as tc:
        with (
            tc.tile_pool(name="sb", bufs=2) as sb,
            tc.tile_pool(name="ps", bufs=3, space="PSUM") as ps,
            tc.tile_pool(name="ps_o", bufs=2, space="PSUM") as ps_o,
        ):
            def load_head(h, split_first):
                kT = sb.tile([128, 2 * L], BF16, tag="kT")
                qT = sb.tile([128, 2 * L], BF16, tag="qT")
                msk = sb.tile([128, 5 * LP], BF16, tag="msk")
                v5 = sb.tile([128, NB * 325], BF16, tag="v5")
                if split_first:
                    nc.sync.dma_start(kT[0:64, :], k_d[h, 0:64, :])
                    if FIRST_Q_SWDGE:
                        # SWDGE generation runs on the (idle) Pool engine in
                        # parallel with the HWDGE generation of the k load
                        nc.gpsimd.dma_start(qT[0:64, :], q_d[h, 0:64, :])
                    else:
                        nc.sync.dma_start(qT[0:64, :], q_d[h, 0:64, :])
                    nc.sync.dma_start(msk[:, 0:LP], m_d[h, :, 0:LP])
                    nc.sync.dma_start(kT[64:128, :], k_d[h, 64:128, :])
                    nc.sync.dma_start(qT[64:128, :], q_d[h, 64:128, :])
                    for c in range(1, 5):
                        nc.sync.dma_start(msk[:, LP * c:LP * (c + 1)],
                                          m_d[h, :, LP * c:LP * (c + 1)])
                else:
                    nc.sync.dma_start(kT, k_d[h])
                    nc.sync.dma_start(qT, q_d[h])
                    nc.sync.dma_start(msk, m_d[h])
                nc.sync.dma_start(v5, v_d[h])
                o_sb = sb.tile([128, NB * 320], BF16, tag="o_sb")
                return dict(kT=kT, qT=qT, msk=msk, v5=v5, o_sb=o_sb, h=h)

            def s_chunk(R, b, c, dest=None, off=0):
                # dest/off: write into a shared tile at a column offset
                # (matmul pieces never cross a 512-f32 PSUM bank boundary)
                pr = 64 * (b // 2)
                xo = L * (b % 2)
                cnt = CNT[c]
                if dest is None:
                    sp = ps.tile([128, LP], F32, tag="sp",
                                 bufs=(2 if M13 else 3))
                else:
                    sp = dest
                bk = 512 * ((off + 511) // 512)
                if bk == off:
                    bk = off + 512
                lhs = R["kT"][pr:pr + 64, xo + KS[c]:xo + KS[c] + cnt]
                nc.tensor.matmul(sp[0:cnt, off:bk], lhs,
                                 R["qT"][pr:pr + 64, xo:xo + (bk - off)],
                                 start=True, stop=True)
                nc.tensor.matmul(sp[0:cnt, bk:off + L], lhs,
                                 R["qT"][pr:pr + 64, xo + (bk - off):xo + L],
                                 start=True, stop=True)
                return sp

            def mask_chunk(R, a_t, sp, c):
                # emit the PSUM->SBUF path for chunk c right after its S
                if c == 0:
                    nc.vector.tensor_tensor(
                        out=a_t[0:121, 0:L], in0=sp[0:121, 0:L],
                        in1=R["msk"][0:121, 0:L], op=Alu.mult)
                elif c in (1, 2, 3):
                    nc.scalar.activation(a_t[0:120, LP * c:LP * c + L],
                                         sp[0:120, 0:L], Act.Copy)
                else:
                    if A4 > 0:
                        nc.scalar.activation(a_t[0:96, 4 * LP:4 * LP + A4],
                                             sp[0:96, 0:A4], Act.Copy)
                    nc.vector.tensor_tensor(
                        out=a_t[0:96, 4 * LP + A4:4 * LP + L],
                        in0=sp[0:96, A4:L],
                        in1=R["msk"][0:96, 4 * LP + A4:4 * LP + L],
                        op=Alu.mult)

            def av_group(Rj, j_b, a_t, o_ps, lc):
                for idx, c in enumerate(AV_ORDER):
                    nc.tensor.matmul(
                        o_ps[0:LW[lc], 65 * lc:65 * lc + 65],
                        a_t[0:CNT[c], LP * c + 128 * lc:LP * c + 128 * lc + LW[lc]],
                        Rj["v5"][0:CNT[c], 325 * j_b + 65 * c:325 * j_b + 65 * c + 65],
                        start=(idx == 0), stop=(idx == 4))

            def late_mults(R, a_t, pc):
                # mask multiplies for the Act-copied chunks of pair i
                io_h = a_t[0:120, LP:4 * LP].rearrange(
                    "p (c l) -> p c l", l=LP)[:, :, 0:L - pc]
                mk_h = R["msk"][0:120, LP:4 * LP].rearrange(
                    "p (c l) -> p c l", l=LP)[:, :, 0:L - pc]
                nc.vector.tensor_tensor(out=io_h, in0=io_h, in1=mk_h,
                                        op=Alu.mult)
                if pc > 0:
                    if POOL_MERGE12:
                        io12 = a_t[0:120, LP:3 * LP].rearrange(
                            "p (c l) -> p c l", l=LP)[:, :, L - pc:L]
                        mk12 = R["msk"][0:120, LP:3 * LP].rearrange(
                            "p (c l) -> p c l", l=LP)[:, :, L - pc:L]
                        nc.gpsimd.tensor_tensor(out=io12, in0=io12, in1=mk12,
                                                op=Alu.mult)
                        cs = (3,)
                    else:
                        cs = (1, 2, 3)
                    for c in cs:
                        io_t = a_t[0:120, LP * c + L - pc:LP * c + L]
                        nc.gpsimd.tensor_tensor(
                            out=io_t, in0=io_t,
                            in1=R["msk"][0:120, LP * c + L - pc:LP * c + L],
                            op=Alu.mult)
                if A4 > 0:
                    io4 = a_t[0:96, 4 * LP:4 * LP + A4]
                    nc.vector.tensor_tensor(
                        out=io4, in0=io4,
                        in1=R["msk"][0:96, 4 * LP:4 * LP + A4], op=Alu.mult)

            def finish_pair(Rj, j_b, o_psj, last):
                rz = sb.tile([128, 5], F32, tag="rz")
                zin = o_psj[:, :].rearrange(
                    "p (c d) -> p c d", d=65)[:, :, 64:65]
                if RECIP_ON_ACT and not last:
                    nc.scalar.activation(
                        rz[:, :].rearrange("p (c d) -> p c d", d=1), zin,
                        Act.Reciprocal)
                else:
                    nc.vector.reciprocal(
                        rz[:, :].rearrange("p (c d) -> p c d", d=1), zin)
                in0 = o_psj[:, :].rearrange(
                    "p (c d) -> p c d", d=65)[:, :, 0:64]
                rzb = AP(rz.tensor, 0, [[5, 128], [1, 5], [0, 64]])
                out_ap = Rj["o_sb"][:, 320 * j_b:320 * j_b + 320].rearrange(
                    "p (c d) -> p c d", d=64)
                if NORM_MODE in ("act_pool", "act_dve") and not last:
                    o_c = sb.tile([128, 320], BF16, tag="o_c")
                    nc.scalar.activation(
                        o_c[:, :].rearrange("p (c d) -> p c d", d=64), in0,
                        Act.Copy)
                    eng = nc.gpsimd if NORM_MODE == "act_pool" else nc.vector
                    eng.tensor_tensor(
                        out=out_ap,
                        in0=o_c[:, :].rearrange("p (c d) -> p c d", d=64),
                        in1=rzb, op=Alu.mult)
                else:
                    nc.vector.tensor_tensor(out=out_ap, in0=in0, in1=rzb,
                                            op=Alu.mult)
                eng = nc.gpsimd if (last and LAST_STORE_SWDGE) else nc.sync
                eng.dma_start(
                    o_d[Rj["h"], :, 320 * j_b:320 * j_b + 320],
                    Rj["o_sb"][:, 320 * j_b:320 * j_b + 320])

            pend = None            # (Rj, j_b, a_t_j, o_ps_j)

            pairs = [(h, b) for h in range(NH) for b in range(NB)]
            heads_loaded = [False] * NH
            R_by_head = {}

            def ensure_head(h):
                if not heads_loaded[h]:
                    R_by_head[h] = load_head(h, split_first=(h == 0))
                    heads_loaded[h] = True
                return R_by_head[h]

            if WARM:
                # dummy matmuls keep the PE busy while the first loads are
                # in flight, so the real matmuls start at full p-state
                dsrc = sb.tile([1, 326], BF16, tag="dsrc", bufs=1)
                nc.vector.memset(dsrc[:, :], 0.0)
                dps = ps_o.tile([128, 325], F32, tag="o_ps")
                for _ in range(WARM):
                    nc.tensor.matmul(dps[0:1, 0:325], dsrc[0:1, 0:1],
                                     dsrc[0:1, 0:325], start=True, stop=True)

            ensure_head(0)

            for s, (h, b) in enumerate(pairs):
                R = R_by_head[h]
                if b == 2 and h + 1 < NH:
                    ensure_head(h + 1)

                a_t = sb.tile([128, 5 * LP], BF16, tag="a_t", bufs=AT_BUFS)
                if pend is not None:
                    Rj, j_b, a_tj, o_psj = pend

                m13 = None
                for tok in (SLOT0_SEQ if s == 0 else SLOT_SEQ):
                    if tok[0] == "S":
                        c = int(tok[1])
                        if M13 and c in (1, 3):
                            if m13 is None:
                                m13 = ps.tile([128, 2 * LP], F32, tag="m13",
                                              bufs=1)
                            s_chunk(R, b, c, dest=m13,
                                    off=(0 if c == 1 else LP))
                            if c == 3:
                                # one merged ScalarE copy for chunks 1 and 3
                                nc.scalar.activation(
                                    a_t[0:120, LP:5 * LP].rearrange(
                                        "p (c l) -> p c l",
                                        l=2 * LP)[:, :, 0:L],
                                    m13[0:120, :].rearrange(
                                        "p (c l) -> p c l", l=LP)[:, :, 0:L],
                                    Act.Copy)
                        else:
                            sp = s_chunk(R, b, c)
                            mask_chunk(R, a_t, sp, c)
                    else:
                        if pend is not None:
                            av_group(Rj, j_b, a_tj, o_psj, int(tok[1]))

                if pend is not None:
                    finish_pair(Rj, j_b, o_psj, last=False)

                late_mults(R, a_t, PC0 if s < 2 else (PC if s % 2 == 0 else PC_ODD))

                o_ps = ps_o.tile([128, 325], F32, tag="o_ps",
                                 bufs=(1 if M13 else 2))
                pend = (R, b, a_t, o_ps)

            # ---- drain ----
            Rj, j_b, a_tj, o_psj = pend
            if SPLIT_LAST_STORE:
                for lc in range(4):
                    av_group(Rj, j_b, a_tj, o_psj, lc)
                rz = sb.tile([128, 5], F32, tag="rz")
                o5 = o_psj[:, :].rearrange("p (c d) -> p c d", d=65)
                nc.vector.reciprocal(
                    rz[:, 0:4].rearrange("p (c d) -> p c d", d=1),
                    o5[:, 0:4, 64:65])
                out5 = Rj["o_sb"][:, 320 * j_b:320 * j_b + 320].rearrange(
                    "p (c d) -> p c d", d=64)
                nc.vector.tensor_tensor(
                    out=out5[:, 0:4, :], in0=o5[:, 0:4, 0:64],
                    in1=AP(rz.tensor, 0, [[5, 128], [1, 4], [0, 64]]),
                    op=Alu.mult)
                nc.sync.dma_start(
                    o_d[Rj["h"], :, 320 * j_b:320 * j_b + 256],
                    Rj["o_sb"][:, 320 * j_b:320 * j_b + 256])
                av_group(Rj, j_b, a_tj, o_psj, 4)
                nc.vector.reciprocal(
                    rz[:, 4:5].rearrange("p (c d) -> p c d", d=1),
                    o5[:, 4:5, 64:65])
                nc.vector.tensor_tensor(
                    out=out5[:, 4:5, :], in0=o5[:, 4:5, 0:64],
                    in1=AP(rz.tensor, 4, [[5, 128], [1, 1], [0, 64]]),
                    op=Alu.mult)
                nc.sync.dma_start(
                    o_d[Rj["h"], :, 320 * j_b + 256:320 * j_b + 320],
                    Rj["o_sb"][:, 320 * j_b + 256:320 * j_b + 320])
            else:
                for lc in range(5):
                    av_group(Rj, j_b, a_tj, o_psj, lc)
                finish_pair(Rj, j_b, o_psj, last=True)

    _split_excess_waits(nc)
    return nc


def _get_nc():
    if "nc" not in _CACHE:
        _CACHE["nc"] = _build_bass()
    return _CACHE["nc"]


def _dist_index():
    if "dist" not in _CACHE:
        gi = np.arange(NBX)
        gj = np.arange(NBY)
        di = (gi[:, None, None, None] - gi[None, None, :, None] + NBX) * 2 * NBY
        dj = gj[None, :, None, None] - gj[None, None, None, :] + NBY
        _CACHE["dist"] = (di + dj).reshape(NBX * NBY, NBX * NBY)
    return _CACHE["dist"]


def _host_shard(query, key, value, topological_params):
    """Build the 8 per-core input dicts (slicing / layout / packing)."""
    q = np.asarray(query, dtype=np.float32)
    k = np.asarray(key, dtype=np.float32)
    v = np.asarray(value, dtype=np.float32)
    p = np.asarray(topological_params, dtype=np.float32)

    qs = np.maximum(q * 0.125, 0.0)
    ks = np.maximum(k, 0.0)

    dist = _dist_index()
    absp = np.abs(p)
    msk_all = np.zeros((H, 128, 5 * LP), dtype=ml_dtypes.bfloat16)
    for h in range(H):
        M = np.ones((L, L), dtype=np.float32)
        M[1:, 1:] = np.take(absp[h], dist)      # [q_grid, k_grid]
        MT = M.T                                # [k, l]
        for c in range(5):
            n = CNT[c]
            msk_all[h, 0:n, LP * c:LP * c + L] = MT[KS[c]:KS[c] + n, :]

    def pack_T(x, bs, hs):
        t = x[bs, :, hs, :]                       # [4, L, 3, 64]
        t = t.transpose(2, 0, 3, 1)               # [3, 4, 64, L]
        t = t.reshape(3, 2, 2, 64, L)             # [3, bhi, blo, d, L]
        t = t.transpose(0, 1, 3, 2, 4)            # [3, bhi, d, blo, L]
        return np.ascontiguousarray(
            t.reshape(3, 128, 2 * L)).astype(ml_dtypes.bfloat16)

    in_maps = []
    for u in range(2):            # batch group
        for g in range(4):        # head group
            bs = slice(4 * u, 4 * u + 4)
            hs = slice(3 * g, 3 * g + 3)
            vs = v[bs, :, hs, :]                  # [4, L, 3, 64]
            v_r = np.zeros((3, 128, NB, 5, 65), np.float32)
            for c in range(5):
                n = CNT[c]
                blk = vs[:, KS[c]:KS[c] + n].transpose(2, 1, 0, 3)
                v_r[:, :n, :, c, 0:64] = blk
                v_r[:, :n, :, c, 64] = 1.0
            in_maps.append({
                "kt": pack_T(ks, bs, hs),
                "qt": pack_T(qs, bs, hs),
                "v5": np.ascontiguousarray(
                    v_r.reshape(3, 128, NB * 325)).astype(ml_dtypes.bfloat16),
                "msk": np.ascontiguousarray(msk_all[hs]),
            })
    return in_maps


def kernel(query, key, value, topological_params):
    from concourse import bass_utils
    nc = _get_nc()
    in_maps = _host_shard(query, key, value, topological_params)
    res = bass_utils.run_bass_kernel_spmd(nc, in_maps, core_ids=list(range(8)))
    out = np.empty((B, L, H, D), dtype=np.float32)
    for u in range(2):
        for g in range(4):
            o = res.results[4 * u + g]["o"]          # [3, 128, NB*320] bf16
            o = o.astype(np.float32).reshape(3, 128, NB, 5, 64)
            for lc in range(5):
                lw = LW[lc]
                blk = o[:, 0:lw, :, lc, :]           # [3, lw, 4, 64]
                out[4 * u:4 * u + 4, 128 * lc:128 * lc + lw,
                    3 * g:3 * g + 3, :] = blk.transpose(2, 1, 0, 3)
    return out
